# revision 1
# baseline (speedup 1.0000x reference)
"""Identity (lossless codec roundtrip) kernel for TRN2, 8 NeuronCores.

Full input: features (8, 4096, 1024) float32.  Output == input bit-exactly.
Sharding: batch dim across the 8 cores -> each core copies a (4096, 1024)
f32 shard (16 MiB) from its input DRAM buffer to its output DRAM buffer
with a single HBM->HBM DMA.
"""

import numpy as np

_B, _M, _N = 8, 4096, 1024
_N_CORES = 8

_cached = {}


def _build_program():
    import concourse.bass as bass
    import concourse.mybir as mybir

    nc = bass.Bass()
    x = nc.declare_dram_parameter("x", [_M, _N], mybir.dt.float32, isOutput=False)
    out = nc.declare_dram_parameter("out", [_M, _N], mybir.dt.float32, isOutput=True)

    with nc.Block() as block, nc.semaphore("dma_sem") as dma_sem:

        @block.sync
        def _(sync):
            sync.dma_start(out=out[:], in_=x[:]).then_inc(dma_sem, 16)
            sync.wait_ge(dma_sem, 16)

    return nc


def _run(features: np.ndarray, trace: bool = False):
    """Returns (output, BassKernelResults)."""
    from concourse.bass_utils import run_bass_kernel_spmd

    if "nc" not in _cached:
        _cached["nc"] = _build_program()
    nc = _cached["nc"]

    features = np.ascontiguousarray(np.asarray(features, dtype=np.float32))
    assert features.shape == (_B, _M, _N), features.shape

    in_maps = [{"x": features[i]} for i in range(_N_CORES)]
    res = run_bass_kernel_spmd(nc, in_maps, core_ids=list(range(_N_CORES)), trace=trace)
    out = np.stack([res.results[i]["out"] for i in range(_N_CORES)], axis=0)
    return out, res


def kernel(features: np.ndarray) -> np.ndarray:
    out, _ = _run(features, trace=False)
    return out


# revision 2
# speedup vs baseline: 1.1447x; 1.1447x over previous
"""Identity (lossless codec roundtrip) kernel for TRN2, 8 NeuronCores.

Full input: features (8, 4096, 1024) float32.  Output == input bit-exactly.

Sharding: batch dim across the 8 cores (data parallel, no communication).
Each core copies its (4096, 1024) f32 shard (16 MiB) from the input DRAM
buffer to the output DRAM buffer with a single HBM->HBM DMA on the sync
engine's HWDGE queue — all 16 SDMA engines stream gap-free at ~21 GB/s
each, ~92% of the per-core HBM (stack) bandwidth limit.

Measured HW exec time: ~61 us/core (transfer floor ~47 us + NEFF fixed
overhead).  Bit-exact output.
"""

import numpy as np

_B, _M, _N = 8, 4096, 1024
_N_CORES = 8

_cached = {}


def _build_program():
    import concourse.bass as bass
    import concourse.mybir as mybir

    # Trimmed constructor: no partition-id load, no monotonic sems, no
    # asserts — shaves preamble before the DMA issues.
    nc = bass.Bass(
        enable_partition_id=False,
        monotonic_sem_count=0,
        enable_asserts=False,
    )
    x = nc.declare_dram_parameter("x", [_M, _N], mybir.dt.float32, isOutput=False)
    out = nc.declare_dram_parameter("out", [_M, _N], mybir.dt.float32, isOutput=True)

    # Top-level emission (no Block) skips block entry/exit barriers.
    with nc.semaphore("s0") as s0:
        nc.sync.dma_start(out=out[:], in_=x[:]).then_inc(s0, 16)
        nc.sync.wait_ge(s0, 16)

    return nc


def _run(features: np.ndarray, trace: bool = False):
    """Returns (output, BassKernelResults)."""
    from concourse.bass_utils import run_bass_kernel_spmd

    if "nc" not in _cached:
        _cached["nc"] = _build_program()
    nc = _cached["nc"]

    features = np.ascontiguousarray(np.asarray(features, dtype=np.float32))
    assert features.shape == (_B, _M, _N), features.shape

    in_maps = [{"x": features[i]} for i in range(_N_CORES)]
    res = run_bass_kernel_spmd(nc, in_maps, core_ids=list(range(_N_CORES)), trace=trace)
    out = np.stack([res.results[i]["out"] for i in range(_N_CORES)], axis=0)
    return out, res


def kernel(features: np.ndarray) -> np.ndarray:
    out, _ = _run(features, trace=False)
    return out


# revision 4
# speedup vs baseline: 1.1868x; 1.0368x over previous
"""Identity (lossless codec roundtrip) kernel for TRN2, 8 NeuronCores.

Full input: features (8, 4096, 1024) float32.  Output == input bit-exactly.

Sharding: batch dim across the 8 cores (data parallel, no communication).
Each core copies its (4096, 1024) f32 shard (16 MiB) from the input DRAM
buffer to the output DRAM buffer with a single HBM->HBM DMA on the sync
engine's HWDGE queue — all 16 SDMA engines stream gap-free at ~21 GB/s
each, ~92% of the per-core HBM (stack) bandwidth limit.

Measured HW exec time: ~61 us/core (transfer floor ~47 us + NEFF fixed
overhead).  Bit-exact output.
"""

import numpy as np

_B, _M, _N = 8, 4096, 1024
_N_CORES = 8

_cached = {}


def _ensure_ntff_hook():
    """Best-effort: synthesize antenv.axon_hooks (absent on this image) so
    run_bass_kernel_spmd can NTFF-profile if tracing is requested (e.g. via
    BASS_TRACE=1).  No-op for the untraced fast path if anything is missing."""
    import sys
    import types

    try:
        import antenv.axon_hooks  # noqa: F401

        return
    except ImportError:
        pass
    try:
        from trn_agent_boot.trn_boot import _ntff_profile_via_ctypes

        hook = _ntff_profile_via_ctypes("/opt/axon/libaxon_pjrt.so")
        mod = types.ModuleType("antenv.axon_hooks")
        mod._hook = hook
        mod.get_axon_ntff_profile_hook = lambda: mod._hook
        mod.set_axon_ntff_profile_hook = lambda h: setattr(mod, "_hook", h)
        sys.modules["antenv.axon_hooks"] = mod
        import antenv

        antenv.axon_hooks = mod
    except Exception:
        pass


def _build_program():
    import concourse.bass as bass
    import concourse.mybir as mybir

    # Trimmed constructor: no partition-id load, no monotonic sems, no
    # asserts — shaves preamble before the DMA issues.
    nc = bass.Bass(
        enable_partition_id=False,
        monotonic_sem_count=0,
        enable_asserts=False,
    )
    x = nc.declare_dram_parameter("x", [_M, _N], mybir.dt.float32, isOutput=False)
    out = nc.declare_dram_parameter("out", [_M, _N], mybir.dt.float32, isOutput=True)

    # Top-level emission (no Block) skips block entry/exit barriers.
    with nc.semaphore("s0") as s0:
        nc.sync.dma_start(out=out[:], in_=x[:]).then_inc(s0, 16)
        nc.sync.wait_ge(s0, 16)

    return nc


def _run(features: np.ndarray, trace: bool = False):
    """Returns (output, BassKernelResults)."""
    from concourse.bass_utils import run_bass_kernel_spmd

    _ensure_ntff_hook()
    if "nc" not in _cached:
        _cached["nc"] = _build_program()
    nc = _cached["nc"]

    features = np.ascontiguousarray(np.asarray(features, dtype=np.float32))
    assert features.shape == (_B, _M, _N), features.shape

    in_maps = [{"x": features[i]} for i in range(_N_CORES)]
    res = run_bass_kernel_spmd(nc, in_maps, core_ids=list(range(_N_CORES)), trace=trace)
    out = np.stack([res.results[i]["out"] for i in range(_N_CORES)], axis=0)
    return out, res


def kernel(features: np.ndarray) -> np.ndarray:
    out, _ = _run(features, trace=False)
    return out
